# revision 1
# baseline (speedup 1.0000x reference)
"""Trainium2 Bass kernel for nn_DistanceLoss (retrieval_knn, 5-way 5-shot).

Computation (per reference):
    q  = relu(queries.flat @ W.T + b)          [5600, 1024]
    se = relu(support.flat @ W.T + b)          [1400, 1024]
    d2 = q_sq + s_sq - 2 q @ se.T              [5600, 1400]
    out[q, c] = -mean_t min_{j in class c} sqrt(relu(d2))

Sharding (8 cores):
  - data-parallel over queries: 13 queries (728 rows) per core (padded 100->104)
  - support projection sharded by support rows (175 rows/core), AllGathered
  - min over class = contiguous 280-col chunk (support rows class-sorted on host)

Layout: "transposed activations" — all matmul operands keep the contraction
dim on partitions. Host pre-transposes Q/W/S and supplies them in fp16
(accumulation is fp32 in PSUM; final rel err ~5e-5). Bias is folded in as an
extra contraction row. q_sq/s_sq are folded into the distance matmul as
fp16 hi+lo split pairs (4 rank-1 terms), so PSUM holds d2 directly at fp32
precision. min(sqrt(relu(x))) == sqrt(relu(min(x))), so the min reduction
runs on raw d2 and sqrt touches only [rows, 5].

Schedule: one fused k-sweep computes the support projection and most of the
query projection sharing W tiles; the AllGather of se.T fires right after,
and the remaining query m-tiles run as a tail sweep that keeps the PE busy
(and HAM-warm) while the collective is in flight.
"""

import os
import sys

if "/opt/trn_rl_repo" not in sys.path:
    sys.path.insert(0, "/opt/trn_rl_repo")

import numpy as np

import concourse.bacc as bacc
import concourse.mybir as mybir
import concourse.tile as tile
from concourse.bass_utils import run_bass_kernel_spmd

WAY, SHOT, T = 5, 5, 56
D_IN, D_OUT = 6144, 1024
N_Q, N_S = 100, 25
N_CORES = 8
QPC = 13                 # queries per core (104 padded)
RPC = QPC * T            # 728 query rows per core
NQR = N_CORES * RPC      # 5824 padded query rows
NSR = N_S * T            # 1400 support rows
SPC = NSR // N_CORES     # 175 support rows per core
KT = D_IN // 128         # 48 full k-tiles
GK = 8                   # max k-tiles per group
GROUPS = [1, 1, 2, 4, 8, 8, 12, 12]  # k-tiles per group (sum 48): small
    # first groups so PE starts early; big last group so the last group's
    # query matmuls (post-collective-trigger) cover the AllGather latency
QM_MAIN = 8              # all query m-tiles run in the shared sweep
NCH = RPC // 2           # 364: query-row matmul chunk
CLS = NSR // WAY         # 280 columns per class
MT = (RPC + 127) // 128  # 6 row tiles (5x128 + 88)
SMW = (128, SPC - 128)   # support row-tile widths (128, 47)
SPCP = 176               # SPC padded: allgather payload [1026,176] fp16 is
                         # a 64B multiple (the collective hangs otherwise)

f32 = mybir.dt.float32
f16 = mybir.dt.float16
AF = mybir.ActivationFunctionType
ALU = mybir.AluOpType
AX = mybir.AxisListType

_MODE = os.environ.get("KERNEL_MODE", "full")


def _build_nc():
    nc = bacc.Bacc("TRN2", target_bir_lowering=False, debug=False,
                   num_devices=N_CORES)
    qT = nc.dram_tensor("qT", [D_IN + 1, RPC], f16, kind="ExternalInput")
    wT = nc.dram_tensor("wT", [D_IN + 1, D_OUT], f16, kind="ExternalInput")
    sT = nc.dram_tensor("sT", [D_IN + 1, SPC], f16, kind="ExternalInput")
    mmask = nc.dram_tensor("mmask", [MT * 128, QPC], f32, kind="ExternalInput")
    ident = nc.dram_tensor("ident", [128, 128], f32, kind="ExternalInput")
    onesd = nc.dram_tensor("onesd", [128, NSR], f16, kind="ExternalInput")
    out = nc.dram_tensor("out", [QPC, WAY], f32, kind="ExternalOutput")

    with tile.TileContext(nc) as tc:
        _body(tc, nc, qT, wT, sT, mmask, ident, onesd, out)
    nc.finalize()
    return nc


def _body(tc, nc, qT, wT, sT, mmask, ident, onesd, out):
    persist_ctx = tc.tile_pool(name="persist", bufs=1)
    persist = persist_ctx.__enter__()

    def ptile(shape, name, dtype=f32):
        return persist.tile(shape, dtype, tag=name, name=name)

    # ---- persistent tiles (live across phases) ----
    # fp32 k-group accumulators
    qacc = [ptile([128, RPC], f"qacc{m}") for m in range(8)]
    sacc = [ptile([SMW[sm], D_OUT], f"sacc{sm}") for sm in range(2)]
    # fp16 operands for the distance matmul
    qacc16 = [ptile([128, RPC], f"qacc16_{m}", f16) for m in range(8)]
    ssq_cols = (ptile([128, 1], "ssq0"), ptile([SMW[1], 1], "ssq1"))

    ident_t = ptile([128, 128], "ident_t")
    ones_col = ptile([128, 1], "ones_col", f16)
    qsq_cols = [ptile([128, 1], f"qsqc{mt}") for mt in range(MT)]
    ssq2 = ptile([2, NSR], "ssq2", f16)
    ones2 = ptile([2, NSR], "ones2", f16)
    ssq_hi_row = ptile([1, SPC], "ssq_hi_row", f16)
    ssq_lo_row = ptile([1, SPC], "ssq_lo_row", f16)
    mins = [ptile([128, WAY], f"mins{mt}") for mt in range(MT)]

    # ragged contraction row (bias / ones)
    wr = ptile([1, D_OUT], "wr", f16)
    qr = ptile([1, RPC], "qr", f16)
    sr = ptile([1, SPC], "sr", f16)

    def emit_preamble():
        # constants not needed before ~85us; emitted mid-sweep-A so their
        # small/strided DMAs don't delay the first group loads
        nc.sync.dma_start(out=ident_t[:], in_=ident[:])
        nc.sync.dma_start(out=ones_col[:], in_=onesd[:, 0:1])
        nc.sync.dma_start(out=ones2[:], in_=onesd[0:2, :])
        nc.sync.dma_start(out=wr[:], in_=wT[D_IN:D_IN + 1, :])
        nc.sync.dma_start(out=qr[:], in_=qT[D_IN:D_IN + 1, :])
        nc.sync.dma_start(out=sr[:], in_=sT[D_IN:D_IN + 1, :])

    # ---- q_sq infrastructure: squares persist until the q_sq-column
    # matmuls after the tail sweep ----
    sqt = [ptile([128, RPC], f"sq{m}", f16) for m in range(8)]

    def emit_qsq(m):
        # relu + fp16 cast, then square (summed later per row-tile)
        nc.vector.tensor_scalar_max(qacc16[m][:], qacc[m][:], 0.0)
        nc.scalar.activation(sqt[m][:], qacc16[m][:], AF.Square)

    # ---- sweep-B first-group preload (loaded during sweep A so the PE
    # transitions between sweeps without a DMA wait) ----
    pre_ctx = tc.tile_pool(name="preload", bufs=1)
    prepool = pre_ctx.__enter__()
    wpre = prepool.tile([128, 2, D_OUT], f16, tag="wpre", name="wpre")
    qpre = prepool.tile([128, 2, RPC], f16, tag="qpre", name="qpre")

    def emit_preload():
        nc.sync.dma_start(
            out=wpre[:],
            in_=wT[0:256, :].rearrange("(g p) d -> p g d", p=128))
        nc.sync.dma_start(
            out=qpre[:],
            in_=qT[0:256, :].rearrange("(g p) d -> p g d", p=128))

    # ---- allgather buffers (the collective fires inside the last k-group,
    # before that group's query matmuls, to start it as early as possible) ----
    dram_ctx = tc.tile_pool(name="dram", bufs=1, space="DRAM")
    dram = dram_ctx.__enter__()
    ag_in = dram.tile([D_OUT + 2, SPCP], f16, tag="ag_in", name="ag_in")
    ag_out = dram.tile([N_CORES, D_OUT + 2, SPCP], f16, tag="ag_out",
                       name="ag_out",
                       addr_space="Local" if _MODE == "nocc" else "Shared")

    # ---- sweep A: support projection only (k-grouped, W+S streamed).
    # Finishing support early lets the AllGather fire at ~55us and hide
    # completely under the query sweep, robust to collective-time variance.
    with (
        tc.tile_pool(name="wspool", bufs=3) as wspool,
        tc.tile_pool(name="sspool", bufs=3) as sspool,
        tc.tile_pool(name="ps", bufs=4, space="PSUM") as pspool,
        tc.tile_pool(name="ssq_scratch", bufs=2) as scratch_pool,
        tc.tile_pool(name="setl", bufs=1) as setl_pool,
        tc.tile_pool(name="ptr", bufs=4, space="PSUM") as ptr_pool,
    ):
        def emit_support_gather():
            # transpose the scaled local se into se.T columns, ship to DRAM,
            # and fire the AllGather
            for j in range(8):
                setl = setl_pool.tile([128, SPC], f16, tag=f"setl{j}",
                                      name=f"setl{j}")
                for sm in range(2):
                    mw = SMW[sm]
                    ptr = ptr_pool.tile([128, 128], f32, tag="ptr",
                                        name="ptr")
                    nc.tensor.transpose(
                        ptr[:, :mw],
                        sacc[sm][:, j * 128:(j + 1) * 128],
                        ident_t[:mw, :mw],
                    )
                    nc.vector.tensor_copy(setl[:, sm * 128:sm * 128 + mw],
                                          ptr[:, :mw])
                nc.sync.dma_start(out=ag_in[j * 128:(j + 1) * 128, 0:SPC],
                                  in_=setl[:])
            for sm in range(2):
                mw = SMW[sm]
                ptr = ptr_pool.tile([128, 128], f32, tag="ptr", name="ptr")
                nc.tensor.transpose(ptr[:1, :mw], ssq_cols[sm][:mw, :],
                                    ident_t[:mw, :mw])
                osl = slice(sm * 128, sm * 128 + mw)
                nc.vector.tensor_copy(ssq_hi_row[:, osl], ptr[:1, :mw])
                nc.vector.tensor_sub(ssq_lo_row[:, osl], ptr[:1, :mw],
                                     ssq_hi_row[:, osl])
            nc.sync.dma_start(out=ag_in[D_OUT:D_OUT + 1, 0:SPC],
                              in_=ssq_hi_row[:])
            nc.sync.dma_start(out=ag_in[D_OUT + 1:D_OUT + 2, 0:SPC],
                              in_=ssq_lo_row[:])
            if _MODE == "nocc":
                for c in range(N_CORES):
                    nc.sync.dma_start(out=ag_out[c], in_=ag_in[:])
            else:
                nc.gpsimd.collective_compute(
                    "AllGather",
                    ALU.bypass,
                    replica_groups=[list(range(N_CORES))],
                    ins=[ag_in[:]],
                    outs=[ag_out[:]],
                )

        SGROUPS = [1, 1, 2, 2, 4, 6, 8, 8, 8, 8]
        kstart = 0
        for g in range(len(SGROUPS)):
            kts = list(range(kstart, kstart + SGROUPS[g]))
            kstart += SGROUPS[g]
            last = g == len(SGROUPS) - 1
            nk = len(kts)
            k0 = kts[0]
            if g == 2:
                emit_preamble()
                emit_preload()
            wg = wspool.tile([128, nk, D_OUT], f16, tag="ws", name=f"ws{g}")
            nc.sync.dma_start(
                out=wg[:],
                in_=wT[k0 * 128:k0 * 128 + nk * 128, :]
                .rearrange("(g p) d -> p g d", p=128))
            sg = sspool.tile([128, nk, SPC], f16, tag="ss", name=f"ss{g}")
            nc.sync.dma_start(
                out=sg[:],
                in_=sT[k0 * 128:k0 * 128 + nk * 128, :]
                .rearrange("(g p) d -> p g d", p=128))

            for sm in range(2):
                mw = SMW[sm]
                msl = slice(sm * 128, sm * 128 + mw)
                for n in range(2):
                    nsl = slice(n * 512, (n + 1) * 512)
                    pst = pspool.tile([128, 512], f32, tag="ps", name="pst")
                    for i, kt in enumerate(kts):
                        nc.tensor.matmul(
                            pst[:mw, :],
                            sg[:, i, msl],
                            wg[:, i, nsl],
                            start=(i == 0),
                            stop=(i == nk - 1 and not last),
                        )
                    if last:
                        nc.tensor.matmul(
                            pst[:mw, :],
                            sr[:, msl],
                            wr[:, nsl],
                            start=False, stop=True,
                        )
                    if g == 0:
                        nc.vector.tensor_copy(sacc[sm][:, nsl], pst[:mw, :])
                    else:
                        nc.vector.tensor_add(sacc[sm][:, nsl],
                                             sacc[sm][:, nsl], pst[:mw, :])

            if last:
                # support epilogue: sacc = -2*relu(raw) = min(-2*raw, 0);
                # s_sq = sum(relu(raw)^2) = sum((0.5*sacc)^2) via ACT accum
                for sm in range(2):
                    mw = SMW[sm]
                    nc.vector.tensor_scalar(sacc[sm][:], sacc[sm][:],
                                            -2.0, 0.0, ALU.mult, ALU.min)
                    sc = scratch_pool.tile([128, D_OUT], f32, tag="ssq_sc",
                                           name="ssq_sc")
                    nc.scalar.activation(sc[:mw, :], sacc[sm][:], AF.Square,
                                         scale=0.5,
                                         accum_out=ssq_cols[sm][:mw, :])
                emit_support_gather()

    # ---- sweep B: query projection (all m-tiles), overlaps the AllGather
    with (
        tc.tile_pool(name="wpool", bufs=2) as wpool,
        tc.tile_pool(name="qpool", bufs=2) as qpool,
        tc.tile_pool(name="pq", bufs=4, space="PSUM") as pqpool,
    ):
        QGROUPS = [2, 4, 6, 8, 8, 8, 8, 4]
        kstart = 0
        for g in range(len(QGROUPS)):
            kts = list(range(kstart, kstart + QGROUPS[g]))
            kstart += QGROUPS[g]
            last = g == len(QGROUPS) - 1
            nk = len(kts)
            k0 = kts[0]
            if g == 0:
                wg, qg = wpre, qpre
            else:
                wg = wpool.tile([128, nk, D_OUT], f16, tag="w", name=f"w{g}")
                nc.sync.dma_start(
                    out=wg[:],
                    in_=wT[k0 * 128:k0 * 128 + nk * 128, :]
                    .rearrange("(g p) d -> p g d", p=128))
                qg = qpool.tile([128, nk, RPC], f16, tag="q", name=f"q{g}")
                nc.sync.dma_start(
                    out=qg[:],
                    in_=qT[k0 * 128:k0 * 128 + nk * 128, :]
                    .rearrange("(g p) d -> p g d", p=128))

            for m in range(8):
                msl = slice(m * 128, (m + 1) * 128)
                for n in range(2):
                    nsl = slice(n * NCH, (n + 1) * NCH)
                    pqt = pqpool.tile([128, NCH], f32, tag="pq", name="pqt")
                    for i, kt in enumerate(kts):
                        nc.tensor.matmul(
                            pqt[:],
                            wg[:, i, msl],
                            qg[:, i, nsl],
                            start=(i == 0),
                            stop=(i == nk - 1 and not last),
                        )
                    if last:
                        nc.tensor.matmul(
                            pqt[:],
                            wr[:, msl],
                            qr[:, nsl],
                            start=False, stop=True,
                        )
                    if g == 0:
                        nc.vector.tensor_copy(qacc[m][:, nsl], pqt[:])
                    else:
                        nc.vector.tensor_add(qacc[m][:, nsl],
                                             qacc[m][:, nsl], pqt[:])
            if last:
                for m in range(8):
                    emit_qsq(m)

    # ---- q_sq columns: qsq_col[mt][r] = sum_dout q^2, via sq.T @ ones ----
    with tc.tile_pool(name="pqsqc", bufs=2, space="PSUM") as pqsqc:
        for mt in range(MT):
            mw = min(128, RPC - mt * 128)
            msl = slice(mt * 128, mt * 128 + mw)
            pq1 = pqsqc.tile([128, 1], f32, tag="pqsqc", name="pqsqc")
            for j in range(8):
                nc.tensor.matmul(pq1[:mw, :], sqt[j][:, msl], ones_col[:],
                                 start=(j == 0), stop=(j == 7))
            nc.vector.tensor_copy(qsq_cols[mt][:mw, :], pq1[:mw, :])

    # ---- phase 2: distance + per-class min + mean ----
    with (
        tc.tile_pool(name="seTp", bufs=1) as seT_pool,
        tc.tile_pool(name="mk", bufs=1) as mk_pool,
        tc.tile_pool(name="pd", bufs=7, space="PSUM") as pd_pool,
        tc.tile_pool(name="po", bufs=1, space="PSUM") as po_pool,
        tc.tile_pool(name="outs", bufs=1) as outs_pool,
    ):
        seT = []
        for j in range(8):
            t_ = seT_pool.tile([128, NSR], f16, tag=f"seT{j}", name=f"seT{j}")
            seT.append(t_)
            nc.sync.dma_start(
                out=t_[:].rearrange("p (c f) -> p c f", c=N_CORES),
                in_=ag_out[:, j * 128:(j + 1) * 128, 0:SPC]
                .rearrange("c p f -> p c f"))
        nc.sync.dma_start(
            out=ssq2[:].rearrange("p (c f) -> p c f", c=N_CORES),
            in_=ag_out[:, D_OUT:D_OUT + 2, 0:SPC].rearrange("c p f -> p c f"))

        for mt in range(MT):
            nc.vector.memset(mins[mt][:], 0.0)

        mkt = []
        for mt in range(MT):
            t_ = mk_pool.tile([128, QPC], f32, tag=f"mk{mt}", name=f"mk{mt}")
            mkt.append(t_)
            nc.gpsimd.dma_start(out=t_[:],
                                in_=mmask[mt * 128:(mt + 1) * 128, :])

        for mt in range(MT):
            mw = min(128, RPC - mt * 128)
            msl = slice(mt * 128, mt * 128 + mw)
            for ch in range(WAY):
                nsl = slice(ch * CLS, (ch + 1) * CLS)
                pd = pd_pool.tile([128, CLS], f32, tag="pd", name="pd")
                for j in range(8):
                    nc.tensor.matmul(
                        pd[:mw, :],
                        qacc16[j][:, msl],
                        seT[j][:, nsl],
                        start=(j == 0), stop=False,
                    )
                nc.tensor.matmul(pd[:mw, :], ones2[:, msl],
                                 ssq2[:, nsl], start=False, stop=True)
                nc.vector.tensor_reduce(
                    mins[mt][:mw, ch:ch + 1], pd[:mw, :],
                    axis=AX.X, op=ALU.min)
            # d2 = min(-2 q.se + s_sq) + q_sq, clamped at 0, then sqrt
            nc.vector.tensor_scalar(mins[mt][:mw, :], mins[mt][:mw, :],
                                    qsq_cols[mt][:mw, :], 0.0,
                                    ALU.add, ALU.max)
            nc.scalar.activation(mins[mt][:], mins[mt][:], AF.Sqrt)

        po = po_pool.tile([QPC, WAY], f32, tag="po", name="po")
        for mt in range(MT):
            nc.tensor.matmul(po[:], mkt[mt][:], mins[mt][:],
                             start=(mt == 0), stop=(mt == MT - 1))
        out_s = outs_pool.tile([QPC, WAY], f32, tag="out_s", name="out_s")
        nc.vector.tensor_copy(out_s[:], po[:])
        nc.sync.dma_start(out=out[:], in_=out_s[:])

    dram_ctx.__exit__(None, None, None)
    pre_ctx.__exit__(None, None, None)
    persist_ctx.__exit__(None, None, None)


_NC_CACHE = {}


def _get_nc():
    if "nc" not in _NC_CACHE:
        _NC_CACHE["nc"] = _build_nc()
    return _NC_CACHE["nc"]


def make_in_maps(support_set, support_labels, queries, clsW_w, clsW_b):
    support_set = np.asarray(support_set, dtype=np.float32)
    support_labels = np.asarray(support_labels)
    queries = np.asarray(queries, dtype=np.float32)
    clsW_w = np.asarray(clsW_w, dtype=np.float32)
    clsW_b = np.asarray(clsW_b, dtype=np.float32)

    # class-sort support rows so each class is a contiguous 280-column block
    perm = np.argsort(support_labels, kind="stable")
    S = support_set[perm].reshape(NSR, D_IN)

    STa = np.empty((D_IN + 1, NSR), np.float16)
    STa[:D_IN] = S.T.astype(np.float16)
    STa[D_IN] = 1.0

    Qp = np.zeros((NQR, D_IN), np.float32)
    Qp[:N_Q * T] = queries.reshape(N_Q * T, D_IN)
    QTa = np.empty((D_IN + 1, NQR), np.float16)
    QTa[:D_IN] = Qp.T.astype(np.float16)
    QTa[D_IN] = 1.0

    WTa = np.empty((D_IN + 1, D_OUT), np.float16)
    WTa[:D_IN] = clsW_w.T.astype(np.float16)
    WTa[D_IN] = clsW_b.astype(np.float16)

    mmask = np.zeros((MT * 128, QPC), np.float32)
    r = np.arange(RPC)
    mmask[r, r // T] = -1.0 / T

    ident = np.eye(128, dtype=np.float32)
    onesd = np.ones((128, NSR), np.float16)

    in_maps = []
    for c in range(N_CORES):
        in_maps.append({
            "qT": np.ascontiguousarray(QTa[:, c * RPC:(c + 1) * RPC]),
            "wT": WTa,
            "sT": np.ascontiguousarray(STa[:, c * SPC:(c + 1) * SPC]),
            "mmask": mmask,
            "ident": ident,
            "onesd": onesd,
        })
    return in_maps


def kernel(support_set, support_labels, queries, clsW_w, clsW_b):
    in_maps = make_in_maps(support_set, support_labels, queries, clsW_w,
                           clsW_b)
    nc = _get_nc()
    res = run_bass_kernel_spmd(nc, in_maps, list(range(N_CORES)))
    out = np.concatenate([res.results[c]["out"] for c in range(N_CORES)], 0)
    return np.ascontiguousarray(out[:N_Q]).astype(np.float32)



# revision 6
# speedup vs baseline: 1.4301x; 1.4301x over previous
"""Trainium2 Bass kernel for nn_DistanceLoss (retrieval_knn, 5-way 5-shot).

Computation (per reference):
    q  = relu(queries.flat @ W.T + b)          [5600, 1024]
    se = relu(support.flat @ W.T + b)          [1400, 1024]
    d2 = q_sq + s_sq - 2 q @ se.T              [5600, 1400]
    out[q, c] = -mean_t min_{j in class c} sqrt(relu(d2))

Sharding (8 cores):
  - data-parallel over queries: 13 queries (728 rows) per core (padded 100->104)
  - support projection sharded by support rows (175 rows/core), AllGathered
  - min over class = contiguous 280-col chunk (support rows class-sorted on host)

All heavy matmuls run in fp8e4 (e4m3) with DoubleRow perf mode: two k-tiles
per instruction at 0.5 cycles/col. Scaling keeps everything in e4m3 range:
host supplies 8*Q, 64*W, 8*S in fp8. The projection PSUM then holds
512*(x.w); scalar-engine activation (Relu, scale=1/64, bias=8b) produces the
fp8 distance operands 8*q / 8*se directly (bias folds into the activation
for q where bias is per-partition; for se a K=1 fp8 matmul adds it in PSUM).
The distance matmul computes (8q)*(-16s) = 64*(-2qs), so q_sq/s_sq are kept
at 64x scale and the final output is unscaled via mmask = -1/(8T). s_sq is
folded into the distance PSUM via a rank-4 fp8 matmul with a 256/16/1
residual decomposition (exact to ~2 scaled units). Both distance operands
are the *quantized* vectors (s_sq computed from fp8-rounded se, q_sq from
fp8-rounded q), so d2 is a true squared distance of the quantized vectors;
min(sqrt(relu(x))) == sqrt(relu(min(x))) lets the min run on raw d2.

Schedule: sweep A computes the support projection (PSUM-resident
accumulation over all 24 k-pairs), the AllGather of the fp8 se.T payload
fires right after, and the query sweep B runs under the collective.
"""

import os
import sys

if "/opt/trn_rl_repo" not in sys.path:
    sys.path.insert(0, "/opt/trn_rl_repo")

import numpy as np
import ml_dtypes

import concourse.bacc as bacc
import concourse.mybir as mybir
import concourse.tile as tile
from concourse.bass_utils import run_bass_kernel_spmd

WAY, SHOT, T = 5, 5, 56
D_IN, D_OUT = 6144, 1024
N_Q, N_S = 100, 25
N_CORES = 8
QPC = 13                 # queries per core (104 padded)
RPC = QPC * T            # 728 query rows per core
RPCP = 736               # RPC padded so fp8 DoubleRow k-slot stride %16 == 0
NQR = N_CORES * RPC      # 5824 padded query rows
NSR = N_S * T            # 1400 support rows
SPC = NSR // N_CORES     # 175 support rows per core
SPCP = 176               # SPC padded (stride %16, and 64B-multiple payload)
KT = D_IN // 128         # 48 k-tiles
KP = KT // 2             # 24 DoubleRow k-pairs
NCH = RPC // 2           # 364: query-row matmul chunk
CLS = NSR // WAY         # 280 columns per class
MT = (RPC + 127) // 128  # 6 row tiles (5x128 + 88)
SMW = (128, SPC - 128)   # support row-tile widths (128, 47)
AGR = D_OUT + 4          # allgather rows: 1024 seT + hi/mid/lo/zero s_sq
                         # [1028,176] fp8 = 180928 B, a 64B multiple
SQA = (192.0, 8.0, 0.25, 0.0)  # s_sq residual-decomposition coefficients

f32 = mybir.dt.float32
f16 = mybir.dt.float16
f8 = mybir.dt.float8e4
AF = mybir.ActivationFunctionType
ALU = mybir.AluOpType
AX = mybir.AxisListType
DR = mybir.MatmulPerfMode.DoubleRow

_MODE = os.environ.get("KERNEL_MODE", "full")


def _build_nc():
    nc = bacc.Bacc("TRN2", target_bir_lowering=False, debug=False,
                   num_devices=N_CORES)
    qT = nc.dram_tensor("qT", [D_IN, RPCP], f8, kind="ExternalInput")
    wT = nc.dram_tensor("wT", [D_IN + 1, D_OUT], f8, kind="ExternalInput")
    sT = nc.dram_tensor("sT", [D_IN + 1, SPCP], f8, kind="ExternalInput")
    bias8 = nc.dram_tensor("bias8", [128, 8], f32, kind="ExternalInput")
    mmask = nc.dram_tensor("mmask", [MT * 128, QPC], f32, kind="ExternalInput")
    identh = nc.dram_tensor("identh", [128, 128], f16, kind="ExternalInput")
    ident32 = nc.dram_tensor("ident32", [128, 128], f32, kind="ExternalInput")
    onescol = nc.dram_tensor("onescol", [128, 1], f16, kind="ExternalInput")
    ones4 = nc.dram_tensor("ones4", [4, RPCP], f8, kind="ExternalInput")
    out = nc.dram_tensor("out", [QPC, WAY], f32, kind="ExternalOutput")

    with tile.TileContext(nc) as tc:
        _body(tc, nc, qT, wT, sT, bias8, mmask, identh, ident32, onescol,
              ones4, out)
    nc.finalize()
    return nc


def _body(tc, nc, qT, wT, sT, bias8, mmask, identh, ident32, onescol,
          ones4, out):
    persist_ctx = tc.tile_pool(name="persist", bufs=1)
    persist = persist_ctx.__enter__()

    def ptile(shape, name, dtype=f32):
        return persist.tile(shape, dtype, tag=name, name=name)

    # ---- persistent tiles (live across phases) ----
    qacc = [ptile([128, RPC], f"qacc{m}") for m in range(8)]
    # fp8 distance operands, DoubleRow pair layout [128, 2, RPCP]
    qacc8 = [ptile([128, 2, RPCP], f"qacc8_{jj}", f8) for jj in range(4)]
    sqt = [ptile([128, RPC], f"sq{m}", f16) for m in range(8)]
    se8 = [ptile([SMW[sm], D_OUT], f"se8_{sm}", f8) for sm in range(2)]
    se16q = [ptile([SMW[sm], D_OUT], f"se16q_{sm}", f16) for sm in range(2)]
    ssq_cols = [ptile([SMW[sm], 1], f"ssq{sm}") for sm in range(2)]

    identh_t = ptile([128, 128], "identh_t", f16)
    ident32_t = ptile([128, 128], "ident32_t")
    onescol_t = ptile([128, 1], "onescol_t", f16)
    bias8_t = ptile([128, 8], "bias8_t")
    ones4_t = ptile([4, RPCP], "ones4_t", f8)
    qsq_cols = [ptile([128, 1], f"qsqc{mt}") for mt in range(MT)]
    ssq4 = ptile([4, N_CORES * SPCP], "ssq4", f8)
    mins = [ptile([128, WAY], f"mins{mt}") for mt in range(MT)]

    # s_sq row decomposition scratch (single partition)
    ssq_row = ptile([1, SPCP], "ssq_row")
    res_row = ptile([1, SPCP], "res_row")
    tmp_row = ptile([1, SPCP], "tmp_row")
    hi_row = ptile([1, SPCP], "hi_row", f8)
    mid_row = ptile([1, SPCP], "mid_row", f8)
    lo_row = ptile([1, SPCP], "lo_row", f8)
    z_row = ptile([1, SPCP], "z_row", f8)

    # ragged contraction row (support bias)
    wr = ptile([1, D_OUT], "wr", f8)
    sr = ptile([1, SPCP], "sr", f8)

    def emit_preamble():
        # constants not needed until sweep A's epilogue; emitted mid-sweep
        nc.sync.dma_start(out=identh_t[:], in_=identh[:])
        nc.sync.dma_start(out=ident32_t[:], in_=ident32[:])
        nc.sync.dma_start(out=onescol_t[:], in_=onescol[:])
        nc.sync.dma_start(out=bias8_t[:], in_=bias8[:])
        nc.sync.dma_start(out=ones4_t[:], in_=ones4[:])
        nc.sync.dma_start(out=wr[:], in_=wT[D_IN:D_IN + 1, :])
        nc.sync.dma_start(out=sr[:], in_=sT[D_IN:D_IN + 1, :])

    # ---- sweep-B first-group preload (loaded during sweep A) ----
    QGROUPS = [3, 3, 4, 4, 5, 5]  # k-pairs per group (sum 24)
    pre_ctx = tc.tile_pool(name="preload", bufs=1)
    prepool = pre_ctx.__enter__()
    np0 = QGROUPS[0]
    wpre = prepool.tile([128, 2 * np0, D_OUT], f8, tag="wpre", name="wpre")
    qpre = prepool.tile([128, 2 * np0, RPCP], f8, tag="qpre", name="qpre")

    def emit_preload():
        nc.sync.dma_start(
            out=wpre[:],
            in_=wT[0:2 * np0 * 128, :].rearrange("(g p) d -> p g d", p=128))
        nc.sync.dma_start(
            out=qpre[:],
            in_=qT[0:2 * np0 * 128, :].rearrange("(g p) d -> p g d", p=128))

    # ---- allgather buffers ----
    dram_ctx = tc.tile_pool(name="dram", bufs=1, space="DRAM")
    dram = dram_ctx.__enter__()
    ag_in = dram.tile([AGR, SPCP], f8, tag="ag_in", name="ag_in")
    ag_out = dram.tile([N_CORES, AGR, SPCP], f8, tag="ag_out",
                       name="ag_out",
                       addr_space="Local" if _MODE == "nocc" else "Shared")

    # ---- sweep A: support projection, PSUM-resident over all 24 k-pairs ----
    with (
        tc.tile_pool(name="wspool", bufs=3) as wspool,
        tc.tile_pool(name="sspool", bufs=3) as sspool,
        tc.tile_pool(name="psA", bufs=1, space="PSUM") as psA,
        tc.tile_pool(name="sq_scratch", bufs=1) as scratch_pool,
        tc.tile_pool(name="setl", bufs=2) as setl_pool,
        tc.tile_pool(name="ptr", bufs=2, space="PSUM") as ptr_pool,
    ):
        psS = [[psA.tile([SMW[sm], 512], f32, tag=f"psS{sm}{n}",
                         name=f"psS{sm}{n}") for n in range(2)]
               for sm in range(2)]

        def emit_support_gather():
            # relu+scale to fp8 (8*se), fp16 copy for transposes, s_sq accum
            for sm in range(2):
                mw = SMW[sm]
                for n in range(2):
                    nsl = slice(n * 512, (n + 1) * 512)
                    nc.scalar.activation(se8[sm][:, nsl], psS[sm][n][:, :],
                                         AF.Relu, scale=1.0 / 64.0)
                nc.vector.tensor_copy(se16q[sm][:], se8[sm][:])
                sc = scratch_pool.tile([128, D_OUT], f16, tag="sq_sc",
                                       name="sq_sc")
                nc.scalar.activation(sc[:mw, :], se16q[sm][:], AF.Square,
                                     accum_out=ssq_cols[sm][:])
            # transpose 8*se (fp16), write -16*se fp8 columns, ship to DRAM
            for j in range(8):
                setl = setl_pool.tile([128, SPCP], f8, tag="setl",
                                      name=f"setl{j}")
                nc.vector.memset(setl[:, 128:SPCP], 0.0)
                for sm in range(2):
                    mw = SMW[sm]
                    ptr = ptr_pool.tile([128, 128], f16, tag="ptr",
                                        name="ptr")
                    nc.tensor.transpose(
                        ptr[:, :mw],
                        se16q[sm][:, j * 128:(j + 1) * 128],
                        identh_t[:mw, :mw],
                    )
                    nc.vector.tensor_scalar(
                        setl[:, sm * 128:sm * 128 + mw], ptr[:, :mw],
                        -2.0, None, ALU.mult)
                nc.sync.dma_start(out=ag_in[j * 128:(j + 1) * 128, :],
                                  in_=setl[:])
            # s_sq rows: transpose cols to a row, decompose 256/16/1 in fp8
            nc.vector.memset(ssq_row[:], 0.0)
            for sm in range(2):
                mw = SMW[sm]
                ptr = ptr_pool.tile([128, 128], f32, tag="ptr32",
                                    name="ptr32")
                nc.tensor.transpose(ptr[:1, :mw], ssq_cols[sm][:mw, :],
                                    ident32_t[:mw, :mw])
                nc.vector.tensor_copy(
                    ssq_row[:, sm * 128:sm * 128 + mw], ptr[:1, :mw])
            nc.vector.tensor_scalar(hi_row[:], ssq_row[:], 1.0 / SQA[0],
                                    None, ALU.mult)
            nc.vector.tensor_scalar(tmp_row[:], hi_row[:], SQA[0],
                                    None, ALU.mult)
            nc.vector.tensor_sub(res_row[:], ssq_row[:], tmp_row[:])
            nc.vector.tensor_scalar(mid_row[:], res_row[:], 1.0 / SQA[1],
                                    None, ALU.mult)
            nc.vector.tensor_scalar(tmp_row[:], mid_row[:], SQA[1],
                                    None, ALU.mult)
            nc.vector.tensor_sub(res_row[:], res_row[:], tmp_row[:])
            nc.vector.tensor_copy(lo_row[:], res_row[:])
            nc.vector.memset(z_row[:], 0.0)
            nc.sync.dma_start(out=ag_in[D_OUT:D_OUT + 1, :], in_=hi_row[:])
            nc.sync.dma_start(out=ag_in[D_OUT + 1:D_OUT + 2, :],
                              in_=mid_row[:])
            nc.sync.dma_start(out=ag_in[D_OUT + 2:D_OUT + 3, :],
                              in_=lo_row[:])
            nc.sync.dma_start(out=ag_in[D_OUT + 3:D_OUT + 4, :],
                              in_=z_row[:])
            if _MODE == "nocc":
                for c in range(N_CORES):
                    nc.sync.dma_start(out=ag_out[c], in_=ag_in[:])
            else:
                nc.gpsimd.collective_compute(
                    "AllGather",
                    ALU.bypass,
                    replica_groups=[list(range(N_CORES))],
                    ins=[ag_in[:]],
                    outs=[ag_out[:]],
                )

        SGROUPS = [2, 3, 4, 4, 5, 6]  # k-pairs per group (sum 24)
        p0 = 0
        for g in range(len(SGROUPS)):
            npr = SGROUPS[g]
            last = g == len(SGROUPS) - 1
            if g == 2:
                emit_preamble()
                emit_preload()
            wg = wspool.tile([128, 2 * npr, D_OUT], f8, tag="ws",
                             name=f"ws{g}")
            nc.sync.dma_start(
                out=wg[:],
                in_=wT[p0 * 256:p0 * 256 + npr * 256, :]
                .rearrange("(g p) d -> p g d", p=128))
            sg = sspool.tile([128, 2 * npr, SPCP], f8, tag="ss",
                             name=f"ss{g}")
            nc.sync.dma_start(
                out=sg[:],
                in_=sT[p0 * 256:p0 * 256 + npr * 256, :]
                .rearrange("(g p) d -> p g d", p=128))

            for ip in range(npr):
                ksl = slice(2 * ip, 2 * ip + 2)
                for sm in range(2):
                    mw = SMW[sm]
                    msl = slice(sm * 128, sm * 128 + mw)
                    for n in range(2):
                        nsl = slice(n * 512, (n + 1) * 512)
                        nc.tensor.matmul(
                            psS[sm][n][:, :],
                            sg[:, ksl, msl],
                            wg[:, ksl, nsl],
                            start=(g == 0 and ip == 0),
                            stop=False,
                            perf_mode=DR,
                        )
            p0 += npr
            if last:
                # support bias via K=1 fp8 matmul: sr(8) * wr(64b) = 512b
                for sm in range(2):
                    mw = SMW[sm]
                    msl = slice(sm * 128, sm * 128 + mw)
                    for n in range(2):
                        nsl = slice(n * 512, (n + 1) * 512)
                        nc.tensor.matmul(
                            psS[sm][n][:, :],
                            sr[:, msl],
                            wr[:, nsl],
                            start=False, stop=True,
                        )
                emit_support_gather()

    # ---- sweep B: query projection (fp8 DoubleRow, SBUF accumulation) ----
    with (
        tc.tile_pool(name="wpool", bufs=2) as wpool,
        tc.tile_pool(name="qpool", bufs=2) as qpool,
        tc.tile_pool(name="pq", bufs=4, space="PSUM") as pqpool,
    ):
        def emit_qdist(m):
            # fp8 distance operand 8*q = relu(qacc/64 + 8b), then its square
            jj, i = m // 2, m % 2
            nc.scalar.activation(qacc8[jj][:, i, 0:RPC], qacc[m][:],
                                 AF.Relu, scale=1.0 / 64.0,
                                 bias=bias8_t[:, m:m + 1])
            nc.scalar.activation(sqt[m][:], qacc8[jj][:, i, 0:RPC],
                                 AF.Square)

        p0 = 0
        for g in range(len(QGROUPS)):
            npr = QGROUPS[g]
            last = g == len(QGROUPS) - 1
            if g == 0:
                wg, qg = wpre, qpre
            else:
                wg = wpool.tile([128, 2 * npr, D_OUT], f8, tag="w",
                                name=f"w{g}")
                nc.sync.dma_start(
                    out=wg[:],
                    in_=wT[p0 * 256:p0 * 256 + npr * 256, :]
                    .rearrange("(g p) d -> p g d", p=128))
                qg = qpool.tile([128, 2 * npr, RPCP], f8, tag="q",
                                name=f"q{g}")
                nc.sync.dma_start(
                    out=qg[:],
                    in_=qT[p0 * 256:p0 * 256 + npr * 256, :]
                    .rearrange("(g p) d -> p g d", p=128))

            for m in range(8):
                msl = slice(m * 128, (m + 1) * 128)
                for n in range(2):
                    nsl = slice(n * NCH, (n + 1) * NCH)
                    pqt = pqpool.tile([128, NCH], f32, tag="pq", name="pqt")
                    for ip in range(npr):
                        ksl = slice(2 * ip, 2 * ip + 2)
                        nc.tensor.matmul(
                            pqt[:],
                            wg[:, ksl, msl],
                            qg[:, ksl, nsl],
                            start=(ip == 0),
                            stop=(ip == npr - 1),
                            perf_mode=DR,
                        )
                    if g == 0:
                        nc.vector.tensor_copy(qacc[m][:, nsl], pqt[:])
                    else:
                        nc.vector.tensor_add(qacc[m][:, nsl],
                                             qacc[m][:, nsl], pqt[:])
                if last:
                    emit_qdist(m)
            p0 += npr

    # ---- q_sq columns: qsq_col[mt][r] = 64*sum q^2, via sq.T @ ones ----
    with tc.tile_pool(name="pqsqc", bufs=2, space="PSUM") as pqsqc:
        for mt in range(MT):
            mw = min(128, RPC - mt * 128)
            msl = slice(mt * 128, mt * 128 + mw)
            pq1 = pqsqc.tile([128, 1], f32, tag="pqsqc", name="pqsqc")
            for j in range(8):
                nc.tensor.matmul(pq1[:mw, :], sqt[j][:, msl], onescol_t[:],
                                 start=(j == 0), stop=(j == 7))
            nc.vector.tensor_copy(qsq_cols[mt][:mw, :], pq1[:mw, :])

    # ---- phase 2: distance + per-class min + mean ----
    with (
        tc.tile_pool(name="seTp", bufs=1) as seT_pool,
        tc.tile_pool(name="mk", bufs=1) as mk_pool,
        tc.tile_pool(name="pd", bufs=7, space="PSUM") as pd_pool,
        tc.tile_pool(name="po", bufs=1, space="PSUM") as po_pool,
        tc.tile_pool(name="outs", bufs=1) as outs_pool,
    ):
        seT = []
        for jj in range(4):
            t_ = seT_pool.tile([128, 2, N_CORES * SPCP], f8, tag=f"seT{jj}",
                               name=f"seT{jj}")
            seT.append(t_)
            for i in range(2):
                j = 2 * jj + i
                nc.sync.dma_start(
                    out=t_[:, i, 0:NSR].rearrange("p (c f) -> p c f",
                                                  c=N_CORES),
                    in_=ag_out[:, j * 128:(j + 1) * 128, 0:SPC]
                    .rearrange("c p f -> p c f"))
        nc.sync.dma_start(
            out=ssq4[:, 0:NSR].rearrange("p (c f) -> p c f", c=N_CORES),
            in_=ag_out[:, D_OUT:D_OUT + 4, 0:SPC].rearrange("c p f -> p c f"))

        for mt in range(MT):
            nc.vector.memset(mins[mt][:], 0.0)

        mkt = []
        for mt in range(MT):
            t_ = mk_pool.tile([128, QPC], f32, tag=f"mk{mt}", name=f"mk{mt}")
            mkt.append(t_)
            nc.gpsimd.dma_start(out=t_[:],
                                in_=mmask[mt * 128:(mt + 1) * 128, :])

        for mt in range(MT):
            mw = min(128, RPC - mt * 128)
            msl = slice(mt * 128, mt * 128 + mw)
            for ch in range(WAY):
                nsl = slice(ch * CLS, (ch + 1) * CLS)
                pd = pd_pool.tile([128, CLS], f32, tag="pd", name="pd")
                for jj in range(4):
                    nc.tensor.matmul(
                        pd[:mw, :],
                        qacc8[jj][:, :, msl],
                        seT[jj][:, :, nsl],
                        start=(jj == 0), stop=False,
                        perf_mode=DR,
                    )
                nc.tensor.matmul(pd[:mw, :], ones4_t[:, msl],
                                 ssq4[:, nsl], start=False, stop=True)
                nc.vector.tensor_reduce(
                    mins[mt][:mw, ch:ch + 1], pd[:mw, :],
                    axis=AX.X, op=ALU.min)
            # 64*d2 = min(-128 q.s + 64 s_sq) + 64 q_sq, clamp 0, sqrt -> 8d
            nc.vector.tensor_scalar(mins[mt][:mw, :], mins[mt][:mw, :],
                                    qsq_cols[mt][:mw, :], 0.0,
                                    ALU.add, ALU.max)
            nc.scalar.activation(mins[mt][:], mins[mt][:], AF.Sqrt)

        po = po_pool.tile([QPC, WAY], f32, tag="po", name="po")
        for mt in range(MT):
            nc.tensor.matmul(po[:], mkt[mt][:], mins[mt][:],
                             start=(mt == 0), stop=(mt == MT - 1))
        out_s = outs_pool.tile([QPC, WAY], f32, tag="out_s", name="out_s")
        nc.vector.tensor_copy(out_s[:], po[:])
        nc.sync.dma_start(out=out[:], in_=out_s[:])

    dram_ctx.__exit__(None, None, None)
    pre_ctx.__exit__(None, None, None)
    persist_ctx.__exit__(None, None, None)


_NC_CACHE = {}


def _get_nc():
    if "nc" not in _NC_CACHE:
        _NC_CACHE["nc"] = _build_nc()
    return _NC_CACHE["nc"]


F8 = ml_dtypes.float8_e4m3


def make_in_maps(support_set, support_labels, queries, clsW_w, clsW_b):
    support_set = np.asarray(support_set, dtype=np.float32)
    support_labels = np.asarray(support_labels)
    queries = np.asarray(queries, dtype=np.float32)
    clsW_w = np.asarray(clsW_w, dtype=np.float32)
    clsW_b = np.asarray(clsW_b, dtype=np.float32)

    # class-sort support rows so each class is a contiguous 280-column block
    perm = np.argsort(support_labels, kind="stable")
    S = support_set[perm].reshape(NSR, D_IN)

    STa = np.zeros((D_IN + 1, N_CORES * SPCP), F8)
    STa[:D_IN, :].reshape(D_IN, N_CORES, SPCP)[:, :, :SPC] = \
        np.asarray(8.0 * S.T, F8).reshape(D_IN, N_CORES, SPC)
    STa[D_IN] = F8(8.0)

    Qp = np.zeros((N_CORES, RPCP, D_IN), np.float32)
    qrows = queries.reshape(N_Q * T, D_IN)
    for c in range(N_CORES):
        nr = min(RPC, max(0, N_Q * T - c * RPC))
        Qp[c, :nr] = qrows[c * RPC:c * RPC + nr]
    QTa = np.asarray(8.0 * Qp, F8).transpose(2, 0, 1)  # [D_IN, 8, RPCP]

    WTa = np.empty((D_IN + 1, D_OUT), F8)
    WTa[:D_IN] = np.asarray(64.0 * clsW_w.T, F8)
    WTa[D_IN] = np.asarray(64.0 * clsW_b, F8)

    bias8 = np.ascontiguousarray(
        (8.0 * clsW_b).reshape(8, 128).T.astype(np.float32))  # [128, 8]

    mmask = np.zeros((MT * 128, QPC), np.float32)
    r = np.arange(RPC)
    mmask[r, r // T] = -1.0 / (8.0 * T)

    identh = np.eye(128, dtype=np.float16)
    ident32 = np.eye(128, dtype=np.float32)
    onescol = np.ones((128, 1), np.float16)
    ones4 = np.zeros((4, RPCP), F8)
    for i, a in enumerate(SQA):
        ones4[i] = F8(a)

    in_maps = []
    for c in range(N_CORES):
        in_maps.append({
            "qT": np.ascontiguousarray(QTa[:, c]),
            "wT": WTa,
            "sT": np.ascontiguousarray(
                STa.reshape(D_IN + 1, N_CORES, SPCP)[:, c]),
            "bias8": bias8,
            "mmask": mmask,
            "identh": identh,
            "ident32": ident32,
            "onescol": onescol,
            "ones4": ones4,
        })
    return in_maps


def kernel(support_set, support_labels, queries, clsW_w, clsW_b):
    in_maps = make_in_maps(support_set, support_labels, queries, clsW_w,
                           clsW_b)
    nc = _get_nc()
    res = run_bass_kernel_spmd(nc, in_maps, list(range(N_CORES)))
    out = np.concatenate([res.results[c]["out"] for c in range(N_CORES)], 0)
    return np.ascontiguousarray(out[:N_Q]).astype(np.float32)


# revision 14
# speedup vs baseline: 1.4909x; 1.0426x over previous
"""Trainium2 Bass kernel for nn_DistanceLoss (retrieval_knn, 5-way 5-shot).

Computation (per reference):
    q  = relu(queries.flat @ W.T + b)          [5600, 1024]
    se = relu(support.flat @ W.T + b)          [1400, 1024]
    d2 = q_sq + s_sq - 2 q @ se.T              [5600, 1400]
    out[q, c] = -mean_t min_{j in class c} sqrt(relu(d2))

Sharding (8 cores):
  - data-parallel over queries: 13 queries (728 rows) per core (padded 100->104)
  - support projection sharded by support rows (175 rows/core), AllGathered
  - min over class = contiguous 280-col chunk (support rows class-sorted on host)

All heavy matmuls run in fp8e4 (e4m3) with DoubleRow perf mode: two k-tiles
per instruction at 0.5 cycles/col. Scaling keeps everything in e4m3 range:
host supplies 8*Q, 64*W, 8*S in fp8. The projection PSUM then holds
512*(x.w); scalar-engine activation (Relu, scale=1/64, bias=8b) produces the
fp8 distance operands 8*q / 8*se directly (bias folds into the activation
for q where bias is per-partition; for se a K=1 fp8 matmul adds it in PSUM).
The distance matmul computes (8q)*(-16s) = 64*(-2qs), so q_sq/s_sq are kept
at 64x scale and the final output is unscaled via mmask = -1/(8T). s_sq is
folded into the distance PSUM via a rank-4 fp8 matmul with a 256/16/1
residual decomposition (exact to ~2 scaled units). Both distance operands
are the *quantized* vectors (s_sq computed from fp8-rounded se, q_sq from
fp8-rounded q), so d2 is a true squared distance of the quantized vectors;
min(sqrt(relu(x))) == sqrt(relu(min(x))) lets the min run on raw d2.

Schedule: sweep A computes the support projection (PSUM-resident
accumulation over all 24 k-pairs), the AllGather of the fp8 se.T payload
fires right after, and the query sweep B runs under the collective.
"""

import os
import sys

if "/opt/trn_rl_repo" not in sys.path:
    sys.path.insert(0, "/opt/trn_rl_repo")

import numpy as np
import ml_dtypes

import concourse.bacc as bacc
import concourse.mybir as mybir
import concourse.tile as tile
from concourse.bass_utils import run_bass_kernel_spmd

WAY, SHOT, T = 5, 5, 56
D_IN, D_OUT = 6144, 1024
N_Q, N_S = 100, 25
N_CORES = 8
QPC = 13                 # queries per core (104 padded)
RPC = QPC * T            # 728 query rows per core
RPCP = 736               # RPC padded so fp8 DoubleRow k-slot stride %16 == 0
NQR = N_CORES * RPC      # 5824 padded query rows
NSR = N_S * T            # 1400 support rows
SPC = NSR // N_CORES     # 175 support rows per core
SPCP = 176               # SPC padded (stride %16, and 64B-multiple payload)
KT = D_IN // 128         # 48 k-tiles
KP = KT // 2             # 24 DoubleRow k-pairs
NCH = RPC // 2           # 364: query-row matmul chunk
CLS = NSR // WAY         # 280 columns per class
MT = (RPC + 127) // 128  # 6 row tiles (5x128 + 88)
SMW = (128, SPC - 128)   # support row-tile widths (128, 47)
AGR = D_OUT + 4          # allgather rows: 1024 seT + hi/mid/lo/zero s_sq
                         # [1028,176] fp8 = 180928 B, a 64B multiple
SQA = (192.0, 8.0, 0.25, 0.0)  # s_sq residual-decomposition coefficients

f32 = mybir.dt.float32
f16 = mybir.dt.float16
f8 = mybir.dt.float8e4
AF = mybir.ActivationFunctionType
ALU = mybir.AluOpType
AX = mybir.AxisListType
DR = mybir.MatmulPerfMode.DoubleRow

_MODE = os.environ.get("KERNEL_MODE", "full")


def _build_nc():
    nc = bacc.Bacc("TRN2", target_bir_lowering=False, debug=False,
                   num_devices=N_CORES)
    qT = nc.dram_tensor("qT", [D_IN, RPCP], f8, kind="ExternalInput")
    wT = nc.dram_tensor("wT", [D_IN + 1, D_OUT], f8, kind="ExternalInput")
    sT = nc.dram_tensor("sT", [D_IN + 1, SPCP], f8, kind="ExternalInput")
    bias8 = nc.dram_tensor("bias8", [128, 8], f32, kind="ExternalInput")
    mmask = nc.dram_tensor("mmask", [MT * 128, QPC], f32, kind="ExternalInput")
    identh = nc.dram_tensor("identh", [128, 128], f16, kind="ExternalInput")
    ident32 = nc.dram_tensor("ident32", [128, 128], f32, kind="ExternalInput")
    onescol = nc.dram_tensor("onescol", [128, 1], f16, kind="ExternalInput")
    ones4 = nc.dram_tensor("ones4", [4, RPCP], f8, kind="ExternalInput")
    out = nc.dram_tensor("out", [QPC, WAY], f32, kind="ExternalOutput")

    with tile.TileContext(nc) as tc:
        _body(tc, nc, qT, wT, sT, bias8, mmask, identh, ident32, onescol,
              ones4, out)
    nc.finalize()
    return nc


def _body(tc, nc, qT, wT, sT, bias8, mmask, identh, ident32, onescol,
          ones4, out):
    persist_ctx = tc.tile_pool(name="persist", bufs=1)
    persist = persist_ctx.__enter__()

    def ptile(shape, name, dtype=f32):
        return persist.tile(shape, dtype, tag=name, name=name)

    # ---- persistent tiles (live across phases) ----
    # W k-group tiles stay resident in SBUF: loaded once in sweep A,
    # reused by sweep B (halves HBM traffic, unstarves sweep A's DMA)
    KGROUPS = [3, 3, 4, 4, 5, 5]  # k-pairs per group (sum 24)
    wsg = [ptile([128, 2 * npr, D_OUT], f"wsg{g}", f8)
           for g, npr in enumerate(KGROUPS)]
    qacc = [ptile([128, RPC], f"qacc{m}") for m in range(8)]
    # fp8 distance operands, DoubleRow pair layout [128, 2, RPCP]
    qacc8 = [ptile([128, 2, RPCP], f"qacc8_{jj}", f8) for jj in range(4)]
    sqt = [ptile([128, RPC], f"sq{m}", f16) for m in range(8)]
    se8 = [ptile([SMW[sm], D_OUT], f"se8_{sm}", f8) for sm in range(2)]
    se16q = [ptile([SMW[sm], D_OUT], f"se16q_{sm}", f16) for sm in range(2)]
    ssq_cols = [ptile([SMW[sm], 1], f"ssq{sm}") for sm in range(2)]

    identh_t = ptile([128, 128], "identh_t", f16)
    ident32_t = ptile([128, 128], "ident32_t")
    onescol_t = ptile([128, 1], "onescol_t", f16)
    bias8_t = ptile([128, 8], "bias8_t")
    ones4_t = ptile([4, RPCP], "ones4_t", f8)
    qsq_cols = [ptile([128, 1], f"qsqc{mt}") for mt in range(MT)]
    ssq4 = ptile([4, N_CORES * SPCP], "ssq4", f8)
    mins = [ptile([128, WAY], f"mins{mt}") for mt in range(MT)]

    # s_sq row decomposition scratch (single partition)
    ssq_row = ptile([1, SPCP], "ssq_row")
    res_row = ptile([1, SPCP], "res_row")
    tmp_row = ptile([1, SPCP], "tmp_row")
    hi_row = ptile([1, SPCP], "hi_row", f8)
    mid_row = ptile([1, SPCP], "mid_row", f8)
    lo_row = ptile([1, SPCP], "lo_row", f8)
    z_row = ptile([1, SPCP], "z_row", f8)

    # ragged contraction row (support bias)
    wr = ptile([1, D_OUT], "wr", f8)
    sr = ptile([1, SPCP], "sr", f8)

    def emit_preamble():
        # constants not needed until sweep A's epilogue; emitted mid-sweep
        nc.sync.dma_start(out=identh_t[:], in_=identh[:])
        nc.sync.dma_start(out=ident32_t[:], in_=ident32[:])
        nc.sync.dma_start(out=onescol_t[:], in_=onescol[:])
        nc.sync.dma_start(out=bias8_t[:], in_=bias8[:])
        nc.sync.dma_start(out=ones4_t[:], in_=ones4[:])
        nc.sync.dma_start(out=wr[:], in_=wT[D_IN:D_IN + 1, :])
        nc.sync.dma_start(out=sr[:], in_=sT[D_IN:D_IN + 1, :])

    # ---- sweep-B first-group preload (loaded during sweep A) ----
    pre_ctx = tc.tile_pool(name="preload", bufs=1)
    prepool = pre_ctx.__enter__()
    np0 = KGROUPS[0]
    qpre = prepool.tile([128, 2 * np0, RPCP], f8, tag="qpre", name="qpre")

    def emit_preload():
        nc.sync.dma_start(
            out=qpre[:],
            in_=qT[0:2 * np0 * 128, :].rearrange("(g p) d -> p g d", p=128))

    # ---- allgather buffers ----
    dram_ctx = tc.tile_pool(name="dram", bufs=1, space="DRAM")
    dram = dram_ctx.__enter__()
    ag_in = dram.tile([AGR, SPCP], f8, tag="ag_in", name="ag_in")
    ag_out = dram.tile([N_CORES, AGR, SPCP], f8, tag="ag_out",
                       name="ag_out",
                       addr_space="Local" if _MODE == "nocc" else "Shared")

    # ---- sweep A: support projection, PSUM-resident over all 24 k-pairs ----
    with (
        tc.tile_pool(name="sspool", bufs=3) as sspool,
        tc.tile_pool(name="psA", bufs=1, space="PSUM") as psA,
        tc.tile_pool(name="sq_scratch", bufs=1) as scratch_pool,
        tc.tile_pool(name="setl", bufs=2) as setl_pool,
        tc.tile_pool(name="ptr", bufs=2, space="PSUM") as ptr_pool,
    ):
        psS = [[psA.tile([SMW[sm], 512], f32, tag=f"psS{sm}{n}",
                         name=f"psS{sm}{n}") for n in range(2)]
               for sm in range(2)]

        def emit_support_gather():
            # relu+scale to fp8 (8*se), fp16 copy for transposes, s_sq accum
            for sm in range(2):
                mw = SMW[sm]
                for n in range(2):
                    nsl = slice(n * 512, (n + 1) * 512)
                    nc.scalar.activation(se8[sm][:, nsl], psS[sm][n][:, :],
                                         AF.Relu, scale=1.0 / 64.0)
                nc.vector.tensor_copy(se16q[sm][:], se8[sm][:])
                sc = scratch_pool.tile([128, D_OUT], f16, tag="sq_sc",
                                       name="sq_sc")
                nc.scalar.activation(sc[:mw, :], se16q[sm][:], AF.Square,
                                     accum_out=ssq_cols[sm][:])
            # transpose 8*se (fp16), write -16*se fp8 columns, ship to DRAM
            for j in range(8):
                setl = setl_pool.tile([128, SPCP], f8, tag="setl",
                                      name=f"setl{j}")
                nc.vector.memset(setl[:, 128:SPCP], 0.0)
                for sm in range(2):
                    mw = SMW[sm]
                    ptr = ptr_pool.tile([128, 128], f16, tag="ptr",
                                        name="ptr")
                    nc.tensor.transpose(
                        ptr[:, :mw],
                        se16q[sm][:, j * 128:(j + 1) * 128],
                        identh_t[:mw, :mw],
                    )
                    nc.vector.tensor_scalar(
                        setl[:, sm * 128:sm * 128 + mw], ptr[:, :mw],
                        -2.0, None, ALU.mult)
                nc.sync.dma_start(out=ag_in[j * 128:(j + 1) * 128, :],
                                  in_=setl[:])
            # s_sq rows: transpose cols to a row, decompose 256/16/1 in fp8
            nc.vector.memset(ssq_row[:], 0.0)
            for sm in range(2):
                mw = SMW[sm]
                ptr = ptr_pool.tile([128, 128], f32, tag="ptr32",
                                    name="ptr32")
                nc.tensor.transpose(ptr[:1, :mw], ssq_cols[sm][:mw, :],
                                    ident32_t[:mw, :mw])
                nc.vector.tensor_copy(
                    ssq_row[:, sm * 128:sm * 128 + mw], ptr[:1, :mw])
            nc.vector.tensor_scalar(hi_row[:], ssq_row[:], 1.0 / SQA[0],
                                    None, ALU.mult)
            nc.vector.tensor_scalar(tmp_row[:], hi_row[:], SQA[0],
                                    None, ALU.mult)
            nc.vector.tensor_sub(res_row[:], ssq_row[:], tmp_row[:])
            nc.vector.tensor_scalar(mid_row[:], res_row[:], 1.0 / SQA[1],
                                    None, ALU.mult)
            nc.vector.tensor_scalar(tmp_row[:], mid_row[:], SQA[1],
                                    None, ALU.mult)
            nc.vector.tensor_sub(res_row[:], res_row[:], tmp_row[:])
            nc.vector.tensor_copy(lo_row[:], res_row[:])
            nc.vector.memset(z_row[:], 0.0)
            nc.sync.dma_start(out=ag_in[D_OUT:D_OUT + 1, :], in_=hi_row[:])
            nc.sync.dma_start(out=ag_in[D_OUT + 1:D_OUT + 2, :],
                              in_=mid_row[:])
            nc.sync.dma_start(out=ag_in[D_OUT + 2:D_OUT + 3, :],
                              in_=lo_row[:])
            nc.sync.dma_start(out=ag_in[D_OUT + 3:D_OUT + 4, :],
                              in_=z_row[:])
            if _MODE == "nocc":
                for c in range(N_CORES):
                    nc.sync.dma_start(out=ag_out[c], in_=ag_in[:])
            else:
                nc.gpsimd.collective_compute(
                    "AllGather",
                    ALU.bypass,
                    replica_groups=[list(range(N_CORES))],
                    ins=[ag_in[:].rearrange("r f -> (r f)")],
                    outs=[ag_out[:].rearrange("c r f -> (c r f)")],
                )

        p0 = 0
        for g in range(len(KGROUPS)):
            npr = KGROUPS[g]
            last = g == len(KGROUPS) - 1
            if g == 2:
                emit_preamble()
                emit_preload()
            wg = wsg[g]
            nc.sync.dma_start(
                out=wg[:],
                in_=wT[p0 * 256:p0 * 256 + npr * 256, :]
                .rearrange("(g p) d -> p g d", p=128))
            sg = sspool.tile([128, 2 * npr, SPCP], f8, tag="ss",
                             name=f"ss{g}")
            nc.sync.dma_start(
                out=sg[:],
                in_=sT[p0 * 256:p0 * 256 + npr * 256, :]
                .rearrange("(g p) d -> p g d", p=128))

            for ip in range(npr):
                ksl = slice(2 * ip, 2 * ip + 2)
                for sm in range(2):
                    mw = SMW[sm]
                    msl = slice(sm * 128, sm * 128 + mw)
                    for n in range(2):
                        nsl = slice(n * 512, (n + 1) * 512)
                        nc.tensor.matmul(
                            psS[sm][n][:, :],
                            sg[:, ksl, msl],
                            wg[:, ksl, nsl],
                            start=(g == 0 and ip == 0),
                            stop=False,
                            perf_mode=DR,
                        )
            p0 += npr
            if last:
                # support bias via K=1 fp8 matmul: sr(8) * wr(64b) = 512b
                for sm in range(2):
                    mw = SMW[sm]
                    msl = slice(sm * 128, sm * 128 + mw)
                    for n in range(2):
                        nsl = slice(n * 512, (n + 1) * 512)
                        nc.tensor.matmul(
                            psS[sm][n][:, :],
                            sr[:, msl],
                            wr[:, nsl],
                            start=False, stop=True,
                        )
                emit_support_gather()

    # ---- sweep B: query projection (fp8 DoubleRow, SBUF accumulation) ----
    with (
        tc.tile_pool(name="qpool", bufs=2) as qpool,
        tc.tile_pool(name="pq", bufs=4, space="PSUM") as pqpool,
    ):
        def emit_qdist(m):
            # fp8 distance operand 8*q = relu(qacc/64 + 8b), then its square
            jj, i = m // 2, m % 2
            nc.scalar.activation(qacc8[jj][:, i, 0:RPC], qacc[m][:],
                                 AF.Relu, scale=1.0 / 64.0,
                                 bias=bias8_t[:, m:m + 1])
            nc.scalar.activation(sqt[m][:], qacc8[jj][:, i, 0:RPC],
                                 AF.Square)

        p0 = 0
        for g in range(len(KGROUPS)):
            npr = KGROUPS[g]
            last = g == len(KGROUPS) - 1
            wg = wsg[g]
            if g == 0:
                qg = qpre
            else:
                qg = qpool.tile([128, 2 * npr, RPCP], f8, tag="q",
                                name=f"q{g}")
                nc.sync.dma_start(
                    out=qg[:],
                    in_=qT[p0 * 256:p0 * 256 + npr * 256, :]
                    .rearrange("(g p) d -> p g d", p=128))

            for m in range(8):
                msl = slice(m * 128, (m + 1) * 128)
                for n in range(2):
                    nsl = slice(n * NCH, (n + 1) * NCH)
                    pqt = pqpool.tile([128, NCH], f32, tag="pq", name="pqt")
                    for ip in range(npr):
                        ksl = slice(2 * ip, 2 * ip + 2)
                        nc.tensor.matmul(
                            pqt[:],
                            wg[:, ksl, msl],
                            qg[:, ksl, nsl],
                            start=(ip == 0),
                            stop=(ip == npr - 1),
                            perf_mode=DR,
                        )
                    if g == 0:
                        nc.vector.tensor_copy(qacc[m][:, nsl], pqt[:])
                    else:
                        nc.vector.tensor_add(qacc[m][:, nsl],
                                             qacc[m][:, nsl], pqt[:])
                if last:
                    emit_qdist(m)
            p0 += npr

    # ---- q_sq columns: qsq_col[mt][r] = 64*sum q^2, via sq.T @ ones ----
    with tc.tile_pool(name="pqsqc", bufs=2, space="PSUM") as pqsqc:
        for mt in range(MT):
            mw = min(128, RPC - mt * 128)
            msl = slice(mt * 128, mt * 128 + mw)
            pq1 = pqsqc.tile([128, 1], f32, tag="pqsqc", name="pqsqc")
            for j in range(8):
                nc.tensor.matmul(pq1[:mw, :], sqt[j][:, msl], onescol_t[:],
                                 start=(j == 0), stop=(j == 7))
            nc.vector.tensor_copy(qsq_cols[mt][:mw, :], pq1[:mw, :])

    # ---- phase 2: distance + per-class min + mean ----
    with (
        tc.tile_pool(name="seTp", bufs=1) as seT_pool,
        tc.tile_pool(name="mk", bufs=1) as mk_pool,
        tc.tile_pool(name="pd", bufs=7, space="PSUM") as pd_pool,
        tc.tile_pool(name="po", bufs=1, space="PSUM") as po_pool,
        tc.tile_pool(name="outs", bufs=1) as outs_pool,
    ):
        seT = []
        for jj in range(4):
            t_ = seT_pool.tile([128, 2, N_CORES * SPCP], f8, tag=f"seT{jj}",
                               name=f"seT{jj}")
            seT.append(t_)
            for i in range(2):
                j = 2 * jj + i
                nc.sync.dma_start(
                    out=t_[:, i, 0:NSR].rearrange("p (c f) -> p c f",
                                                  c=N_CORES),
                    in_=ag_out[:, j * 128:(j + 1) * 128, 0:SPC]
                    .rearrange("c p f -> p c f"))
        nc.sync.dma_start(
            out=ssq4[:, 0:NSR].rearrange("p (c f) -> p c f", c=N_CORES),
            in_=ag_out[:, D_OUT:D_OUT + 4, 0:SPC].rearrange("c p f -> p c f"))

        for mt in range(MT):
            nc.vector.memset(mins[mt][:], 0.0)

        mkt = []
        for mt in range(MT):
            t_ = mk_pool.tile([128, QPC], f32, tag=f"mk{mt}", name=f"mk{mt}")
            mkt.append(t_)
            nc.gpsimd.dma_start(out=t_[:],
                                in_=mmask[mt * 128:(mt + 1) * 128, :])

        po = po_pool.tile([QPC, WAY], f32, tag="po", name="po")
        for mt in range(MT):
            mw = min(128, RPC - mt * 128)
            msl = slice(mt * 128, mt * 128 + mw)
            # jj-outer so consecutive matmuls reuse the stationary operand
            pds = [pd_pool.tile([128, CLS], f32, tag="pd",
                                name=f"pd{mt}_{ch}") for ch in range(WAY)]
            for jj in range(4):
                for ch in range(WAY):
                    nsl = slice(ch * CLS, (ch + 1) * CLS)
                    nc.tensor.matmul(
                        pds[ch][:mw, :],
                        qacc8[jj][:, :, msl],
                        seT[jj][:, :, nsl],
                        start=(jj == 0), stop=False,
                        perf_mode=DR,
                    )
            for ch in range(WAY):
                nsl = slice(ch * CLS, (ch + 1) * CLS)
                nc.tensor.matmul(pds[ch][:mw, :], ones4_t[:, msl],
                                 ssq4[:, nsl], start=False, stop=True)
                nc.vector.tensor_reduce(
                    mins[mt][:mw, ch:ch + 1], pds[ch][:mw, :],
                    axis=AX.X, op=ALU.min)
            # 64*d2 = min(-128 q.s + 64 s_sq) + 64 q_sq, clamp 0, sqrt -> 8d
            nc.vector.tensor_scalar(mins[mt][:mw, :], mins[mt][:mw, :],
                                    qsq_cols[mt][:mw, :], 0.0,
                                    ALU.add, ALU.max)
            nc.scalar.activation(mins[mt][:], mins[mt][:], AF.Sqrt)
            nc.tensor.matmul(po[:], mkt[mt][:], mins[mt][:],
                             start=(mt == 0), stop=(mt == MT - 1))
        out_s = outs_pool.tile([QPC, WAY], f32, tag="out_s", name="out_s")
        nc.vector.tensor_copy(out_s[:], po[:])
        nc.sync.dma_start(out=out[:], in_=out_s[:])

    dram_ctx.__exit__(None, None, None)
    pre_ctx.__exit__(None, None, None)
    persist_ctx.__exit__(None, None, None)


_NC_CACHE = {}


def _get_nc():
    if "nc" not in _NC_CACHE:
        _NC_CACHE["nc"] = _build_nc()
    return _NC_CACHE["nc"]


F8 = ml_dtypes.float8_e4m3


def make_in_maps(support_set, support_labels, queries, clsW_w, clsW_b):
    support_set = np.asarray(support_set, dtype=np.float32)
    support_labels = np.asarray(support_labels)
    queries = np.asarray(queries, dtype=np.float32)
    clsW_w = np.asarray(clsW_w, dtype=np.float32)
    clsW_b = np.asarray(clsW_b, dtype=np.float32)

    # class-sort support rows so each class is a contiguous 280-column block
    perm = np.argsort(support_labels, kind="stable")
    S = support_set[perm].reshape(NSR, D_IN)

    STa = np.zeros((D_IN + 1, N_CORES * SPCP), F8)
    STa[:D_IN, :].reshape(D_IN, N_CORES, SPCP)[:, :, :SPC] = \
        np.asarray(8.0 * S.T, F8).reshape(D_IN, N_CORES, SPC)
    STa[D_IN] = F8(8.0)

    Qp = np.zeros((N_CORES, RPCP, D_IN), np.float32)
    qrows = queries.reshape(N_Q * T, D_IN)
    for c in range(N_CORES):
        nr = min(RPC, max(0, N_Q * T - c * RPC))
        Qp[c, :nr] = qrows[c * RPC:c * RPC + nr]
    QTa = np.asarray(8.0 * Qp, F8).transpose(2, 0, 1)  # [D_IN, 8, RPCP]

    WTa = np.empty((D_IN + 1, D_OUT), F8)
    WTa[:D_IN] = np.asarray(64.0 * clsW_w.T, F8)
    WTa[D_IN] = np.asarray(64.0 * clsW_b, F8)

    bias8 = np.ascontiguousarray(
        (8.0 * clsW_b).reshape(8, 128).T.astype(np.float32))  # [128, 8]

    mmask = np.zeros((MT * 128, QPC), np.float32)
    r = np.arange(RPC)
    mmask[r, r // T] = -1.0 / (8.0 * T)

    identh = np.eye(128, dtype=np.float16)
    ident32 = np.eye(128, dtype=np.float32)
    onescol = np.ones((128, 1), np.float16)
    ones4 = np.zeros((4, RPCP), F8)
    for i, a in enumerate(SQA):
        ones4[i] = F8(a)

    in_maps = []
    for c in range(N_CORES):
        in_maps.append({
            "qT": np.ascontiguousarray(QTa[:, c]),
            "wT": WTa,
            "sT": np.ascontiguousarray(
                STa.reshape(D_IN + 1, N_CORES, SPCP)[:, c]),
            "bias8": bias8,
            "mmask": mmask,
            "identh": identh,
            "ident32": ident32,
            "onescol": onescol,
            "ones4": ones4,
        })
    return in_maps


def kernel(support_set, support_labels, queries, clsW_w, clsW_b):
    in_maps = make_in_maps(support_set, support_labels, queries, clsW_w,
                           clsW_b)
    nc = _get_nc()
    res = run_bass_kernel_spmd(nc, in_maps, list(range(N_CORES)))
    out = np.concatenate([res.results[c]["out"] for c in range(N_CORES)], 0)
    return np.ascontiguousarray(out[:N_Q]).astype(np.float32)


# revision 25
# speedup vs baseline: 1.6389x; 1.0992x over previous
"""Trainium2 Bass kernel for nn_DistanceLoss (retrieval_knn, 5-way 5-shot).

Computation (per reference):
    q  = relu(queries.flat @ W.T + b)          [5600, 1024]
    se = relu(support.flat @ W.T + b)          [1400, 1024]
    d2 = q_sq + s_sq - 2 q @ se.T              [5600, 1400]
    out[q, c] = -mean_t min_{j in class c} sqrt(relu(d2))

Sharding (8 cores):
  - data-parallel over queries: 13 queries (728 rows) per core (padded 100->104)
  - support projection sharded by support rows (175 rows/core), AllGathered
  - min over class = contiguous 280-col chunk (support rows class-sorted on host)

All heavy matmuls run in fp8e4 (e4m3) with DoubleRow perf mode: two k-tiles
per instruction at 0.5 cycles/col. Scaling keeps everything in e4m3 range:
host supplies 8*Q, 64*W, 8*S in fp8. The projection PSUM then holds
512*(x.w); scalar-engine activation (Relu, scale=1/64, bias=8b) produces the
fp8 distance operands 8*q / 8*se directly (bias folds into the activation
for q where bias is per-partition; for se a K=1 fp8 matmul adds it in PSUM).
The distance matmul computes (8q)*(-16s) = 64*(-2qs), so q_sq/s_sq are kept
at 64x scale and the final output is unscaled via mmask = -1/(8T). s_sq is
folded into the distance PSUM via a rank-4 fp8 matmul with a 256/16/1
residual decomposition (exact to ~2 scaled units). Both distance operands
are the *quantized* vectors (s_sq computed from fp8-rounded se, q_sq from
fp8-rounded q), so d2 is a true squared distance of the quantized vectors;
min(sqrt(relu(x))) == sqrt(relu(min(x))) lets the min run on raw d2.

Schedule: sweep A computes the support projection (PSUM-resident
accumulation over all 24 k-pairs), the AllGather of the fp8 se.T payload
fires right after, and the query sweep B runs under the collective.
"""

import os
import sys

if "/opt/trn_rl_repo" not in sys.path:
    sys.path.insert(0, "/opt/trn_rl_repo")

import numpy as np
import ml_dtypes

import concourse.bacc as bacc
import concourse.mybir as mybir
import concourse.tile as tile
from concourse.bass_utils import run_bass_kernel_spmd

WAY, SHOT, T = 5, 5, 56
D_IN, D_OUT = 6144, 1024
N_Q, N_S = 100, 25
N_CORES = 8
QPC = 13                 # queries per core (104 padded)
RPC = QPC * T            # 728 query rows per core
RPCP = 736               # RPC padded so fp8 DoubleRow k-slot stride %16 == 0
NQR = N_CORES * RPC      # 5824 padded query rows
NSR = N_S * T            # 1400 support rows
SPC = NSR // N_CORES     # 175 support rows per core
SPCP = 176               # SPC padded (stride %16, and 64B-multiple payload)
KT = D_IN // 128         # 48 k-tiles
KP = KT // 2             # 24 DoubleRow k-pairs
NCH = RPC // 2           # 364: query-row matmul chunk
CLS = NSR // WAY         # 280 columns per class
MT = (RPC + 127) // 128  # 6 row tiles (5x128 + 88)
SMW = (128, SPC - 128)   # support row-tile widths (128, 47)
AGR = D_OUT + 4          # allgather rows: 1024 seT + hi/mid/lo/zero s_sq
                         # [1028,176] fp8 = 180928 B, a 64B multiple
SQA = (192.0, 8.0, 0.25, 0.0)  # s_sq residual-decomposition coefficients

f32 = mybir.dt.float32
f16 = mybir.dt.float16
f8 = mybir.dt.float8e4
AF = mybir.ActivationFunctionType
ALU = mybir.AluOpType
AX = mybir.AxisListType
DR = mybir.MatmulPerfMode.DoubleRow

_MODE = os.environ.get("KERNEL_MODE", "full")


def _build_nc():
    nc = bacc.Bacc("TRN2", target_bir_lowering=False, debug=False,
                   num_devices=N_CORES)
    qT = nc.dram_tensor("qT", [D_IN, RPCP], f8, kind="ExternalInput")
    wT = nc.dram_tensor("wT", [D_IN + 1, D_OUT], f8, kind="ExternalInput")
    sT = nc.dram_tensor("sT", [D_IN + 1, SPCP], f8, kind="ExternalInput")
    bias8 = nc.dram_tensor("bias8", [128, 8], f32, kind="ExternalInput")
    mmask = nc.dram_tensor("mmask", [MT * 128, QPC], f32, kind="ExternalInput")
    identh = nc.dram_tensor("identh", [128, 128], f16, kind="ExternalInput")
    ident32 = nc.dram_tensor("ident32", [128, 128], f32, kind="ExternalInput")
    onescol = nc.dram_tensor("onescol", [128, 1], f16, kind="ExternalInput")
    ones4 = nc.dram_tensor("ones4", [4, RPCP], f8, kind="ExternalInput")
    out = nc.dram_tensor("out", [QPC, WAY], f32, kind="ExternalOutput")

    with tile.TileContext(nc) as tc:
        _body(tc, nc, qT, wT, sT, bias8, mmask, identh, ident32, onescol,
              ones4, out)
    nc.finalize()
    return nc


def _body(tc, nc, qT, wT, sT, bias8, mmask, identh, ident32, onescol,
          ones4, out):
    persist_ctx = tc.tile_pool(name="persist", bufs=1)
    persist = persist_ctx.__enter__()

    def ptile(shape, name, dtype=f32):
        return persist.tile(shape, dtype, tag=name, name=name)

    # ---- persistent tiles (live across phases) ----
    # W k-group tiles stay resident in SBUF: loaded once in sweep A,
    # reused by sweep B (halves HBM traffic, unstarves sweep A's DMA)
    KGROUPS = [1, 2, 4, 5, 6, 6]  # k-pairs per group (sum 24)
    wsg = [ptile([128, 2 * npr, D_OUT], f"wsg{g}", f8)
           for g, npr in enumerate(KGROUPS)]
    qacc = [ptile([128, RPC], f"qacc{m}") for m in range(8)]
    # fp8 distance operands, DoubleRow pair layout [128, 2, RPCP]
    qacc8 = [ptile([128, 2, RPCP], f"qacc8_{jj}", f8) for jj in range(4)]
    sqt = [ptile([128, RPC], f"sq{m}", f16) for m in range(8)]
    se8 = [ptile([SMW[sm], D_OUT], f"se8_{sm}", f8) for sm in range(2)]
    se16q = [ptile([SMW[sm], D_OUT], f"se16q_{sm}", f16) for sm in range(2)]
    ssq_cols = [ptile([SMW[sm], 1], f"ssq{sm}") for sm in range(2)]

    identh_t = ptile([128, 128], "identh_t", f16)
    ident32_t = ptile([128, 128], "ident32_t")
    onescol_t = ptile([128, 1], "onescol_t", f16)
    bias8_t = ptile([128, 8], "bias8_t")
    ones4_t = ptile([4, RPCP], "ones4_t", f8)
    qsq_cols = [ptile([128, 1], f"qsqc{mt}") for mt in range(MT)]
    ssq4 = ptile([4, N_CORES * SPCP], "ssq4", f8)
    mins = [ptile([128, WAY], f"mins{mt}") for mt in range(MT)]
    seT = [ptile([128, 2, N_CORES * SPCP], f"seT{jj}", f8) for jj in range(4)]

    # s_sq row decomposition scratch (single partition)
    ssq_row = ptile([1, SPCP], "ssq_row")
    res_row = ptile([1, SPCP], "res_row")
    tmp_row = ptile([1, SPCP], "tmp_row")
    hi_row = ptile([1, SPCP], "hi_row", f8)
    mid_row = ptile([1, SPCP], "mid_row", f8)
    lo_row = ptile([1, SPCP], "lo_row", f8)
    z_row = ptile([1, SPCP], "z_row", f8)

    # ragged contraction row (support bias)
    wr = ptile([1, D_OUT], "wr", f8)
    sr = ptile([1, SPCP], "sr", f8)

    def emit_preamble():
        # constants not needed until sweep A's epilogue; emitted mid-sweep
        nc.sync.dma_start(out=identh_t[:], in_=identh[:])
        nc.sync.dma_start(out=ident32_t[:], in_=ident32[:])
        nc.sync.dma_start(out=onescol_t[:], in_=onescol[:])
        nc.sync.dma_start(out=bias8_t[:], in_=bias8[:])
        nc.sync.dma_start(out=ones4_t[:], in_=ones4[:])
        nc.sync.dma_start(out=wr[:], in_=wT[D_IN:D_IN + 1, :])
        nc.sync.dma_start(out=sr[:], in_=sT[D_IN:D_IN + 1, :])

    # ---- sweep-B first-group preload (loaded during sweep A) ----
    pre_ctx = tc.tile_pool(name="preload", bufs=1)
    prepool = pre_ctx.__enter__()
    np0 = KGROUPS[0]
    qpre = prepool.tile([128, 2 * np0, RPCP], f8, tag="qpre", name="qpre")

    def emit_preload():
        nc.sync.dma_start(
            out=qpre[:],
            in_=qT[0:2 * np0 * 128, :].rearrange("(g p) d -> p g d", p=128))

    # ---- allgather buffers ----
    dram_ctx = tc.tile_pool(name="dram", bufs=1, space="DRAM")
    dram = dram_ctx.__enter__()
    ag_in = dram.tile([AGR, SPCP], f8, tag="ag_in", name="ag_in")
    ag_out = dram.tile([N_CORES, AGR, SPCP], f8, tag="ag_out",
                       name="ag_out",
                       addr_space="Local" if _MODE == "nocc" else "Shared")

    # ---- sweep A: support projection, PSUM-resident over all 24 k-pairs ----
    with (
        tc.tile_pool(name="sspool", bufs=3) as sspool,
        tc.tile_pool(name="psA", bufs=1, space="PSUM") as psA,
        tc.tile_pool(name="sq_scratch", bufs=1) as scratch_pool,
        tc.tile_pool(name="setl", bufs=2) as setl_pool,
        tc.tile_pool(name="ptr", bufs=2, space="PSUM") as ptr_pool,
    ):
        psS = [[psA.tile([SMW[sm], 512], f32, tag=f"psS{sm}{n}",
                         name=f"psS{sm}{n}") for n in range(2)]
               for sm in range(2)]

        def emit_support_gather():
            # relu+scale to fp8 (8*se), fp16 copy for transposes, s_sq accum
            for sm in range(2):
                mw = SMW[sm]
                for n in range(2):
                    nsl = slice(n * 512, (n + 1) * 512)
                    nc.scalar.activation(se8[sm][:, nsl], psS[sm][n][:, :],
                                         AF.Relu, scale=1.0 / 64.0)
                nc.gpsimd.tensor_copy(se16q[sm][:], se8[sm][:])
                sc = scratch_pool.tile([128, D_OUT], f16, tag="sq_sc",
                                       name="sq_sc")
                nc.scalar.activation(sc[:mw, :], se16q[sm][:], AF.Square,
                                     accum_out=ssq_cols[sm][:])
            # transpose 8*se (fp16), write -16*se fp8 columns, ship to DRAM
            for j in range(8):
                setl = setl_pool.tile([128, SPCP], f8, tag="setl",
                                      name=f"setl{j}")
                nc.vector.memset(setl[:, 128:SPCP], 0.0)
                for sm in range(2):
                    mw = SMW[sm]
                    ptr = ptr_pool.tile([128, 128], f16, tag="ptr",
                                        name="ptr")
                    nc.tensor.transpose(
                        ptr[:, :mw],
                        se16q[sm][:, j * 128:(j + 1) * 128],
                        identh_t[:mw, :mw],
                    )
                    nc.vector.tensor_scalar(
                        setl[:, sm * 128:sm * 128 + mw], ptr[:, :mw],
                        -2.0, None, ALU.mult)
                nc.sync.dma_start(out=ag_in[j * 128:(j + 1) * 128, :],
                                  in_=setl[:])
            # s_sq rows: transpose cols to a row, decompose 256/16/1 in fp8
            nc.vector.memset(ssq_row[:], 0.0)
            for sm in range(2):
                mw = SMW[sm]
                ptr = ptr_pool.tile([128, 128], f32, tag="ptr32",
                                    name="ptr32")
                nc.tensor.transpose(ptr[:1, :mw], ssq_cols[sm][:mw, :],
                                    ident32_t[:mw, :mw])
                nc.vector.tensor_copy(
                    ssq_row[:, sm * 128:sm * 128 + mw], ptr[:1, :mw])
            nc.vector.tensor_scalar(hi_row[:], ssq_row[:], 1.0 / SQA[0],
                                    None, ALU.mult)
            nc.vector.tensor_scalar(tmp_row[:], hi_row[:], SQA[0],
                                    None, ALU.mult)
            nc.vector.tensor_sub(res_row[:], ssq_row[:], tmp_row[:])
            nc.vector.tensor_scalar(mid_row[:], res_row[:], 1.0 / SQA[1],
                                    None, ALU.mult)
            nc.vector.tensor_scalar(tmp_row[:], mid_row[:], SQA[1],
                                    None, ALU.mult)
            nc.vector.tensor_sub(res_row[:], res_row[:], tmp_row[:])
            nc.vector.tensor_copy(lo_row[:], res_row[:])
            nc.vector.memset(z_row[:], 0.0)
            nc.sync.dma_start(out=ag_in[D_OUT:D_OUT + 1, :], in_=hi_row[:])
            nc.sync.dma_start(out=ag_in[D_OUT + 1:D_OUT + 2, :],
                              in_=mid_row[:])
            nc.sync.dma_start(out=ag_in[D_OUT + 2:D_OUT + 3, :],
                              in_=lo_row[:])
            nc.sync.dma_start(out=ag_in[D_OUT + 3:D_OUT + 4, :],
                              in_=z_row[:])
            if _MODE == "nocc":
                for c in range(N_CORES):
                    nc.sync.dma_start(out=ag_out[c], in_=ag_in[:])
            else:
                nc.gpsimd.collective_compute(
                    "AllGather",
                    ALU.bypass,
                    replica_groups=[list(range(N_CORES))],
                    ins=[ag_in[:].rearrange("r f -> (r f)")],
                    outs=[ag_out[:].rearrange("c r f -> (c r f)")],
                )
            # unpack DMAs issued now so they fire the moment the AG lands
            # (fully overlapped by sweep B)
            for jj in range(4):
                for i in range(2):
                    j = 2 * jj + i
                    nc.scalar.dma_start(
                        out=seT[jj][:, i, 0:NSR].rearrange(
                            "p (c f) -> p c f", c=N_CORES),
                        in_=ag_out[:, j * 128:(j + 1) * 128, 0:SPC]
                        .rearrange("c p f -> p c f"))
            nc.scalar.dma_start(
                out=ssq4[:, 0:NSR].rearrange("p (c f) -> p c f", c=N_CORES),
                in_=ag_out[:, D_OUT:D_OUT + 4, 0:SPC]
                .rearrange("c p f -> p c f"))

        p0 = 0
        for g in range(len(KGROUPS)):
            npr = KGROUPS[g]
            last = g == len(KGROUPS) - 1
            if g == 2:
                emit_preamble()
                emit_preload()
            wg = wsg[g]
            nc.sync.dma_start(
                out=wg[:],
                in_=wT[p0 * 256:p0 * 256 + npr * 256, :]
                .rearrange("(g p) d -> p g d", p=128))
            sg = sspool.tile([128, 2 * npr, SPCP], f8, tag="ss",
                             name=f"ss{g}")
            nc.sync.dma_start(
                out=sg[:],
                in_=sT[p0 * 256:p0 * 256 + npr * 256, :]
                .rearrange("(g p) d -> p g d", p=128))

            for ip in range(npr):
                ksl = slice(2 * ip, 2 * ip + 2)
                for sm in range(2):
                    mw = SMW[sm]
                    msl = slice(sm * 128, sm * 128 + mw)
                    for n in range(2):
                        nsl = slice(n * 512, (n + 1) * 512)
                        nc.tensor.matmul(
                            psS[sm][n][:, :],
                            sg[:, ksl, msl],
                            wg[:, ksl, nsl],
                            start=(g == 0 and ip == 0),
                            stop=False,
                            perf_mode=DR,
                        )
            p0 += npr
            if last:
                # support bias via K=1 fp8 matmul: sr(8) * wr(64b) = 512b
                for sm in range(2):
                    mw = SMW[sm]
                    msl = slice(sm * 128, sm * 128 + mw)
                    for n in range(2):
                        nsl = slice(n * 512, (n + 1) * 512)
                        nc.tensor.matmul(
                            psS[sm][n][:, :],
                            sr[:, msl],
                            wr[:, nsl],
                            start=False, stop=True,
                        )
                emit_support_gather()

    # ---- sweep B: query projection (fp8 DoubleRow, SBUF accumulation) ----
    with (
        tc.tile_pool(name="qpool", bufs=2) as qpool,
        tc.tile_pool(name="pq", bufs=6, space="PSUM") as pqpool,
    ):
        def emit_qdist(m):
            # fp8 distance operand 8*q = relu(qacc/64 + 8b), then its square
            jj, i = m // 2, m % 2
            nc.scalar.activation(qacc8[jj][:, i, 0:RPC], qacc[m][:],
                                 AF.Relu, scale=1.0 / 64.0,
                                 bias=bias8_t[:, m:m + 1])
            nc.scalar.activation(sqt[m][:], qacc8[jj][:, i, 0:RPC],
                                 AF.Square)

        p0 = 0
        for g in range(len(KGROUPS)):
            npr = KGROUPS[g]
            last = g == len(KGROUPS) - 1
            wg = wsg[g]
            if g == 0:
                qg = qpre
            else:
                qg = qpool.tile([128, 2 * npr, RPCP], f8, tag="q",
                                name=f"q{g}")
                nc.sync.dma_start(
                    out=qg[:],
                    in_=qT[p0 * 256:p0 * 256 + npr * 256, :]
                    .rearrange("(g p) d -> p g d", p=128))

            for m in range(8):
                msl = slice(m * 128, (m + 1) * 128)
                for n in range(2):
                    nsl = slice(n * NCH, (n + 1) * NCH)
                    pqt = pqpool.tile([128, NCH], f32, tag="pq", name="pqt")
                    for ip in range(npr):
                        ksl = slice(2 * ip, 2 * ip + 2)
                        nc.tensor.matmul(
                            pqt[:],
                            wg[:, ksl, msl],
                            qg[:, ksl, nsl],
                            start=(ip == 0),
                            stop=(ip == npr - 1),
                            perf_mode=DR,
                        )
                    if g == 0:
                        nc.vector.tensor_copy(qacc[m][:, nsl], pqt[:])
                    else:
                        nc.vector.tensor_add(qacc[m][:, nsl],
                                             qacc[m][:, nsl], pqt[:])
                if last:
                    emit_qdist(m)
            p0 += npr

    # ---- q_sq columns: qsq_col[mt][r] = 64*sum q^2, via sq.T @ ones ----
    with tc.tile_pool(name="pqsqc", bufs=2, space="PSUM") as pqsqc:
        for mt in range(MT):
            mw = min(128, RPC - mt * 128)
            msl = slice(mt * 128, mt * 128 + mw)
            pq1 = pqsqc.tile([128, 1], f32, tag="pqsqc", name="pqsqc")
            for j in range(8):
                nc.tensor.matmul(pq1[:mw, :], sqt[j][:, msl], onescol_t[:],
                                 start=(j == 0), stop=(j == 7))
            nc.vector.tensor_copy(qsq_cols[mt][:mw, :], pq1[:mw, :])

    # ---- phase 2: distance + per-class min + mean ----
    with (
        tc.tile_pool(name="mk", bufs=1) as mk_pool,
        tc.tile_pool(name="pd", bufs=7, space="PSUM") as pd_pool,
        tc.tile_pool(name="po", bufs=1, space="PSUM") as po_pool,
        tc.tile_pool(name="outs", bufs=1) as outs_pool,
    ):
        for mt in range(MT):
            nc.vector.memset(mins[mt][:], 0.0)

        mkt = []
        for mt in range(MT):
            t_ = mk_pool.tile([128, QPC], f32, tag=f"mk{mt}", name=f"mk{mt}")
            mkt.append(t_)
            nc.gpsimd.dma_start(out=t_[:],
                                in_=mmask[mt * 128:(mt + 1) * 128, :])

        po = po_pool.tile([QPC, WAY], f32, tag="po", name="po")
        for mt in range(MT):
            mw = min(128, RPC - mt * 128)
            msl = slice(mt * 128, mt * 128 + mw)
            # jj-outer so consecutive matmuls reuse the stationary operand
            pds = [pd_pool.tile([128, CLS], f32, tag="pd",
                                name=f"pd{mt}_{ch}") for ch in range(WAY)]
            for jj in range(4):
                for ch in range(WAY):
                    nsl = slice(ch * CLS, (ch + 1) * CLS)
                    nc.tensor.matmul(
                        pds[ch][:mw, :],
                        qacc8[jj][:, :, msl],
                        seT[jj][:, :, nsl],
                        start=(jj == 0), stop=False,
                        perf_mode=DR,
                    )
            for ch in range(WAY):
                nsl = slice(ch * CLS, (ch + 1) * CLS)
                nc.tensor.matmul(pds[ch][:mw, :], ones4_t[:, msl],
                                 ssq4[:, nsl], start=False, stop=True)
                nc.vector.tensor_reduce(
                    mins[mt][:mw, ch:ch + 1], pds[ch][:mw, :],
                    axis=AX.X, op=ALU.min)
            # 64*d2 = min(-128 q.s + 64 s_sq) + 64 q_sq, clamp 0, sqrt -> 8d
            nc.vector.tensor_scalar(mins[mt][:mw, :], mins[mt][:mw, :],
                                    qsq_cols[mt][:mw, :], 0.0,
                                    ALU.add, ALU.max)
            nc.scalar.activation(mins[mt][:], mins[mt][:], AF.Sqrt)
            nc.tensor.matmul(po[:], mkt[mt][:], mins[mt][:],
                             start=(mt == 0), stop=(mt == MT - 1))
        out_s = outs_pool.tile([QPC, WAY], f32, tag="out_s", name="out_s")
        nc.vector.tensor_copy(out_s[:], po[:])
        nc.sync.dma_start(out=out[:], in_=out_s[:])

    dram_ctx.__exit__(None, None, None)
    pre_ctx.__exit__(None, None, None)
    persist_ctx.__exit__(None, None, None)


_NC_CACHE = {}


def _get_nc():
    if "nc" not in _NC_CACHE:
        _NC_CACHE["nc"] = _build_nc()
    return _NC_CACHE["nc"]


F8 = ml_dtypes.float8_e4m3


def make_in_maps(support_set, support_labels, queries, clsW_w, clsW_b):
    support_set = np.asarray(support_set, dtype=np.float32)
    support_labels = np.asarray(support_labels)
    queries = np.asarray(queries, dtype=np.float32)
    clsW_w = np.asarray(clsW_w, dtype=np.float32)
    clsW_b = np.asarray(clsW_b, dtype=np.float32)

    # class-sort support rows so each class is a contiguous 280-column block
    perm = np.argsort(support_labels, kind="stable")
    S = support_set[perm].reshape(NSR, D_IN)

    STa = np.zeros((D_IN + 1, N_CORES * SPCP), F8)
    STa[:D_IN, :].reshape(D_IN, N_CORES, SPCP)[:, :, :SPC] = \
        np.asarray(8.0 * S.T, F8).reshape(D_IN, N_CORES, SPC)
    STa[D_IN] = F8(8.0)

    Qp = np.zeros((N_CORES, RPCP, D_IN), np.float32)
    qrows = queries.reshape(N_Q * T, D_IN)
    for c in range(N_CORES):
        nr = min(RPC, max(0, N_Q * T - c * RPC))
        Qp[c, :nr] = qrows[c * RPC:c * RPC + nr]
    QTa = np.asarray(8.0 * Qp, F8).transpose(2, 0, 1)  # [D_IN, 8, RPCP]

    WTa = np.empty((D_IN + 1, D_OUT), F8)
    WTa[:D_IN] = np.asarray(64.0 * clsW_w.T, F8)
    WTa[D_IN] = np.asarray(64.0 * clsW_b, F8)

    bias8 = np.ascontiguousarray(
        (8.0 * clsW_b).reshape(8, 128).T.astype(np.float32))  # [128, 8]

    mmask = np.zeros((MT * 128, QPC), np.float32)
    r = np.arange(RPC)
    mmask[r, r // T] = -1.0 / (8.0 * T)

    identh = np.eye(128, dtype=np.float16)
    ident32 = np.eye(128, dtype=np.float32)
    onescol = np.ones((128, 1), np.float16)
    ones4 = np.zeros((4, RPCP), F8)
    for i, a in enumerate(SQA):
        ones4[i] = F8(a)

    in_maps = []
    for c in range(N_CORES):
        in_maps.append({
            "qT": np.ascontiguousarray(QTa[:, c]),
            "wT": WTa,
            "sT": np.ascontiguousarray(
                STa.reshape(D_IN + 1, N_CORES, SPCP)[:, c]),
            "bias8": bias8,
            "mmask": mmask,
            "identh": identh,
            "ident32": ident32,
            "onescol": onescol,
            "ones4": ones4,
        })
    return in_maps


def kernel(support_set, support_labels, queries, clsW_w, clsW_b):
    in_maps = make_in_maps(support_set, support_labels, queries, clsW_w,
                           clsW_b)
    nc = _get_nc()
    res = run_bass_kernel_spmd(nc, in_maps, list(range(N_CORES)))
    out = np.concatenate([res.results[c]["out"] for c in range(N_CORES)], 0)
    return np.ascontiguousarray(out[:N_Q]).astype(np.float32)
